# revision 18
# baseline (speedup 1.0000x reference)
"""Trainium2 Bass kernel for nn_CentersDistance (retrieval_knn).

logits[k, n] = -||centers[k] - inputs[n]||^2
             = 2*(centers @ inputs.T)[k, n] - ||centers[k]||^2 - ||inputs[n]||^2

Strategy (8 NeuronCores, data-parallel over the N=8192 inputs):
  * device computes ONLY the cross term 2*c.x as 64 fp8e4m3 DoubleRow
    matmuls per core (contraction 256/MM via the [p, 2, f] interleave,
    2 fp8 MACs/cell/cycle -> 216 ns/MM at 2.4 GHz = the DR stream
    floor), PSUM fp32, copied to SBUF as fp16 and stored; the exact norm
    terms (float64 on host) are added on the host after gather.
  * group order in "quarters": (m0-3,h0), (m0-3,h1), (m4-7,h0),
    (m4-7,h1); within a quarter the matmuls run j-major (all 4 groups'
    DR-step j before step j+1) so the PE starts on the first 128 KB
    chunk pair as soon as it lands.
  * chunk-major data layout: both DRAM and SBUF hold each 128 KB chunk
    with 1 KB contiguous per partition, so every load descriptor is
    1 KB (512 B descriptors measurably cap the two HW-DGE rings at
    ~270 GB/s aggregate under 8-core contention).  ct chunks (j, m-half)
    on the Scalar ring, xt chunks (j, h) on the Sync ring, one
    semaphore per chunk.
  * PSUM->SBUF fp16 copies alternate DVE (even retirement order) / ACT
    (odd); stores alternate Sync (even) / ACT (odd).  PSUM bank-reuse
    waits are hoisted into the previous quarter's last matmul phase so
    they never stall the PE's LDWEIGHTS pull-ahead at a quarter boundary.
  * No final store wait: the NRT postamble's DMA drain covers the last
    in-flight stores.
  * N_WU throwaway matmuls on an uninitialized scratch tile keep the PE
    busy from preamble-end to first-chunk-arrival so the HAM clock gate
    (~3.4 us sustained-busy window) opens before the real matmuls start.
"""

import threading
from contextlib import ExitStack

import numpy as np
import ml_dtypes

import concourse.mybir as mybir
from concourse import bacc
from concourse.bass_utils import run_bass_kernel_spmd

N_CORES = 8
N, K, D = 8192, 1024, 1024
NSH = N // N_CORES  # per-core slab of inputs
P = 128             # SBUF partitions
NF = 512            # matmul moving free dim (one fp32 PSUM bank)

KS = D // P         # 8 contraction subtiles of 128
J = KS // 2         # 4 DoubleRow steps of 256
M_TILES = K // P    # 8 center tiles
H_TILES = NSH // NF # 2 moving-dim tiles

G = M_TILES * H_TILES  # 16 output groups of [128, 512]
N_WU = 7               # full-width PE warm-up matmuls (~427 ns cold each)
N_WU_SHORT = 8         # short (N=128) warm-ups filling until first chunks land

# group order: quarters (m0-3,h0), (m0-3,h1), (m4-7,h0), (m4-7,h1)
GROUP_ORDER = (
    [(m, 0) for m in range(4)]
    + [(m, 1) for m in range(4)]
    + [(m, 0) for m in range(4, 8)]
    + [(m, 1) for m in range(4, 8)]
)

_DT = mybir.dt.float8e4
_NP_DT = ml_dtypes.float8_e4m3

_cache = threading.local()


# chunk-major layout: tensor [128, 16, 512]; chunk c occupies slots
# (2c, 2c+1); slot 2c+s holds contraction subtile (2j+s) for the chunk's
# 512-column slice.  ct chunk c = mhalf*4 + j; xt chunk c = h*4 + j.
def _ct_chunk(j, m):
    return (m // 4) * J + j


def _xt_chunk(j, h):
    return h * J + j


def _build_nc(dt=_DT, n_wu=N_WU):
    nc = bacc.Bacc(
        "TRN2", target_bir_lowering=False, debug=False, num_devices=N_CORES
    )
    ct = nc.dram_tensor("ct", [P, 2 * KS, NF], dt, kind="ExternalInput").ap()
    xt = nc.dram_tensor("xt", [P, 2 * KS, NF], dt, kind="ExternalInput").ap()
    out = nc.dram_tensor("out", [K, NSH], mybir.dt.float16, kind="ExternalOutput").ap()

    out_r = out.rearrange("(m p) n -> m p n", p=P)

    DR = mybir.MatmulPerfMode.DoubleRow

    with (
        nc.sbuf_tensor("wu_sb", [P, NF], dt) as wu_sb,
        nc.sbuf_tensor("ct_sb", [P, 2 * KS, NF], dt) as ct_sb,
        nc.sbuf_tensor("xt_sb", [P, 2 * KS, NF], dt) as xt_sb,
        nc.sbuf_tensor("ot_sb", [P, G * NF], mybir.dt.float16) as ot_sb,
        ExitStack() as stack,
        nc.semaphore("mm_sem") as mm_sem,
        nc.semaphore("dve_cp") as dve_cp,
        nc.semaphore("act_cp") as act_cp,
        nc.semaphore("dma_out") as dma_out,
        nc.Block() as block,
    ):
        ct_sems = [
            stack.enter_context(nc.semaphore(f"ct_sem{i}")) for i in range(2 * J)
        ]
        xt_sems = [
            stack.enter_context(nc.semaphore(f"xt_sem{i}")) for i in range(2 * J)
        ]
        ps = [
            stack.enter_context(nc.psum_tensor(f"ps{b}", [P, NF], mybir.dt.float32))
            for b in range(8)
        ]

        cp_sem = {0: dve_cp, 1: act_cp}  # order parity -> copy engine sem

        @block.sync
        def _(sync):
            for c in range(2 * J):  # xt chunks, consumption order
                sync.dma_start(
                    xt_sb[:, 2 * c : 2 * c + 2, :], xt[:, 2 * c : 2 * c + 2, :]
                ).then_inc(xt_sems[c], 16)
            # even-order stores, gated on the DVE copy
            for o, (m, h) in enumerate(GROUP_ORDER):
                if o % 2 != 0:
                    continue
                sync.wait_ge(dve_cp, (o // 2) + 1)
                sync.dma_start(
                    out_r[m][:, h * NF : (h + 1) * NF],
                    ot_sb[:, o * NF : (o + 1) * NF],
                ).then_inc(dma_out, 16)
            # first half of the split last group (copied by the DVE)
            m15, h15 = GROUP_ORDER[G - 1]
            sync.wait_ge(dve_cp, G // 2 + 1)
            sync.dma_start(
                out_r[m15][:, h15 * NF : h15 * NF + NF // 2],
                ot_sb[:, (G - 1) * NF : (G - 1) * NF + NF // 2],
            ).then_inc(dma_out, 16)

        @block.scalar
        def _(scalar):
            for c in range(2 * J):  # ct chunks, consumption order
                scalar.dma_start(
                    ct_sb[:, 2 * c : 2 * c + 2, :], ct[:, 2 * c : 2 * c + 2, :]
                ).then_inc(ct_sems[c], 16)
            # odd-order copies (PSUM -> SBUF fp16) on ACT, each followed
            # by its own store (gated on the copy's semaphore: the DMA
            # descriptor fetch can race the ACT pipeline's SBUF write).
            # the last group (o = G-1) is split in half across DVE+Sync
            # and ACT+Scalar so the final copy+store tail halves.
            for o, (m, h) in enumerate(GROUP_ORDER):
                if o % 2 != 1 or o == G - 1:
                    continue
                scalar.wait_ge(mm_sem, o + 1)
                nc.scalar.copy(
                    ot_sb[:, o * NF : (o + 1) * NF], ps[o % 8][:]
                ).then_inc(act_cp, 1)
                scalar.wait_ge(act_cp, (o // 2) + 1)
                scalar.dma_start(
                    out_r[m][:, h * NF : (h + 1) * NF],
                    ot_sb[:, o * NF : (o + 1) * NF],
                ).then_inc(dma_out, 16)
            m15, h15 = GROUP_ORDER[G - 1]
            scalar.wait_ge(mm_sem, G)
            nc.scalar.copy(
                ot_sb[:, (G - 1) * NF + NF // 2 : G * NF],
                ps[(G - 1) % 8][:, NF // 2 : NF],
            ).then_inc(act_cp, 1)
            scalar.wait_ge(act_cp, G // 2)
            scalar.dma_start(
                out_r[m15][:, h15 * NF + NF // 2 : (h15 + 1) * NF],
                ot_sb[:, (G - 1) * NF + NF // 2 : G * NF],
            ).then_inc(dma_out, 16)

        @block.tensor
        def _(tensor):
            # warm-up: keep the PE busy from preamble-end to first-chunk
            # arrival so the HAM clock gate opens early; the tail uses
            # short (N=128) matmuls so the overshoot past data-arrival is
            # at 107 ns granularity.  wu_sb is deliberately uninitialized;
            # bank 7 is rewritten with start=True by order-7's first
            # matmul much later.
            for _ in range(n_wu):
                nc.tensor.matmul(
                    ps[7][:], wu_sb[:, 0:P], wu_sb[:], start=True, stop=True
                )
            for _ in range(N_WU_SHORT if n_wu else 0):
                nc.tensor.matmul(
                    ps[7][:, 0:P], wu_sb[:, 0:P], wu_sb[:, 0:P],
                    start=True, stop=True,
                )
            ct_waited = set()
            xt_waited = set()
            for q in range(4):  # quarter
                quarter = list(enumerate(GROUP_ORDER))[4 * q : 4 * q + 4]
                for j in range(J):
                    for i, (o, (m, h)) in enumerate(quarter):
                        ci = _ct_chunk(j, m)
                        if ci not in ct_waited:
                            ct_waited.add(ci)
                            tensor.wait_ge(ct_sems[ci], 16)
                        xi = _xt_chunk(j, h)
                        if xi not in xt_waited:
                            xt_waited.add(xi)
                            tensor.wait_ge(xt_sems[xi], 16)
                        if j == J - 2 and q >= 1:
                            # hoisted PSUM bank-reuse wait for the NEXT
                            # quarter's order (o_next = o+4): its bank was
                            # last drained by order o_next-8's copy, done
                            # well before this point, so the wait is free
                            # here but would stall the LDWEIGHTS
                            # pull-ahead at the quarter boundary
                            o_next = 4 * (q + 1) + i
                            if o_next < G and o_next >= 8:
                                tensor.wait_ge(
                                    cp_sem[(o_next - 8) % 2],
                                    ((o_next - 8) // 2) + 1,
                                )
                        mm = nc.tensor.matmul(
                            ps[o % 8][:],
                            ct_sb[
                                :,
                                2 * _ct_chunk(j, m) : 2 * _ct_chunk(j, m) + 2,
                                (m % 4) * P : (m % 4 + 1) * P,
                            ],
                            xt_sb[:, 2 * _xt_chunk(j, h) : 2 * _xt_chunk(j, h) + 2, :],
                            start=(j == 0),
                            stop=(j == J - 1),
                            perf_mode=DR,
                        )
                        if j == J - 1:
                            mm.then_inc(mm_sem, 1)

        @block.vector
        def _(vector):
            # even-order copies (PSUM -> SBUF fp16) on the DVE, plus the
            # first half of the split last group
            for o, (m, h) in enumerate(GROUP_ORDER):
                if o % 2 != 0:
                    continue
                vector.wait_ge(mm_sem, o + 1)
                nc.vector.tensor_copy(
                    ot_sb[:, o * NF : (o + 1) * NF], ps[o % 8][:]
                ).then_inc(dve_cp, 1)
            vector.wait_ge(mm_sem, G)
            nc.vector.tensor_copy(
                ot_sb[:, (G - 1) * NF : (G - 1) * NF + NF // 2],
                ps[(G - 1) % 8][:, 0 : NF // 2],
            ).then_inc(dve_cp, 1)

    nc.compile()
    return nc


def _get_nc():
    if not hasattr(_cache, "nc"):
        _cache.nc = _build_nc()
    return _cache.nc


def _pack_chunk_major(a_t):
    """[D, F] (F = 1024) -> [128, 16, 512] chunk-major fp8 layout.

    Slot 2c+s of the output holds contraction subtile 2j+s, column half
    fhalf, where c = fhalf*4 + j.  Contraction index d = ks*128 + p.
    """
    Dd, F = a_t.shape
    v = a_t.reshape(J, 2, P, 2, NF)        # [j, s, p, fhalf, f']
    v = v.transpose(2, 3, 0, 1, 4)         # [p, fhalf, j, s, f']
    return np.ascontiguousarray(v.reshape(P, 2 * KS, NF))


def kernel(inputs, centers, _trace=False):
    inputs = np.asarray(inputs, dtype=np.float32)
    centers = np.asarray(centers, dtype=np.float32)

    csq = np.sum(centers.astype(np.float64) ** 2, axis=1)
    xsq = np.sum(inputs.astype(np.float64) ** 2, axis=1)

    ct = _pack_chunk_major(centers.T.astype(_NP_DT))
    xt2 = (2.0 * inputs).T.astype(_NP_DT)

    in_maps = []
    for i in range(N_CORES):
        sl = slice(i * NSH, (i + 1) * NSH)
        in_maps.append(
            {"ct": ct, "xt": _pack_chunk_major(xt2[:, sl])}
        )

    nc = _get_nc()
    try:
        res = run_bass_kernel_spmd(
            nc, in_maps, core_ids=list(range(N_CORES)), trace=_trace
        )
    except ModuleNotFoundError:
        # NTFF trace glue is absent in some images; rerun without tracing
        res = run_bass_kernel_spmd(
            nc, in_maps, core_ids=list(range(N_CORES)), trace=False
        )
    if _trace:
        kernel.last_results = res

    # device returns the raw cross term [K, NSH] per core; add the exact
    # norm terms on the host
    cross = np.concatenate(
        [r["out"] for r in res.results], axis=1
    ).astype(np.float32)
    logits = cross - csq[:, None].astype(np.float32)
    logits -= xsq[None, :].astype(np.float32)
    return logits
